# revision 6
# baseline (speedup 1.0000x reference)
"""Trainium2 Bass kernel for elementwise i1e(z) = exp(-|z|) * I1(z),
z in [0.1, 50], shape (32, 4096, 1024) f32, data-parallel over 8 cores.

Fast path (memory-regime):
    HBM I/O in fp16 (halves DMA traffic; tolerance is 2e-2).
    w' = k / sqrt(z + B)      (1 ACT pass: Dsqrt(s*z + s*B) = 0.5/sqrt(.))
    out = ((((C0 - w')*w' + C1)*w' + C2)*w' + C3)*w'
                              (1 custom DVE pass, depth-8, 4 constants;
                               quintic with negative leading coeff -k^5)
Fit: minimax-relative deg-5 odd-free poly of i1e in w = rsqrt(z+B),
B=2.448; max rel err ~6.3e-3 (fit) / ~7e-3 (incl. fp16 quantization).
"""
import numpy as np

NCORES = 8
NT, P, FD = 32, 128, 4096          # per-core: 32 tiles of [128, 4096] fp16
FULL_SHAPE = (32, 4096, 1024)
PER_CORE = (4, 4096, 1024)

# --- approximation constants (see module docstring) ---
_B = 2.4480
_C15 = [0.193748, 3.346473, -19.009253, 47.355799, -41.558003]  # c1..c5
_K = float((-_C15[4]) ** 0.2)                 # 2.10732...
# AbsRsqrt(s*z + s*B) = 1/sqrt(s) * rsqrt(z+B) = k*rsqrt(z+B), k=1/sqrt(s)
_SCALE = float(1.0 / _K**2)
_BIAS = float(_B * _SCALE)
_C0 = float(_C15[3] / _K**4)                  # coeff of w'^4 (s0)
_C1 = float(_C15[2] / _K**3)                  # coeff of w'^3 (s1)
_C2 = float(_C15[1] / _K**2)                  # coeff of w'^2 (imm2)
_C3 = float(_C15[0] / _K)                     # coeff of w'^1 (in1 spill)

_state = {}


def _register_ops():
    import concourse.dve_ops as dve_ops
    from concourse.dve_spec import (
        Spec, Src0, C0, C1, C2, C3, _spill_c3_to_src1, lower, _has_src1,
    )
    from concourse.dve_uop import DveOpSpec

    if "IVE_P5" in dve_ops._SUB_OPCODE_FOR_NAME:
        return {o.name: o for o in dve_ops.OPS}

    f32 = np.float32

    def ref_p5(in0, in1, s0, s1, imm2):
        c3 = np.asarray(in1, f32).reshape(-1, 1)
        x = in0.astype(f32)
        return (((((s0 - x) * x + s1) * x + imm2) * x + c3) * x).astype(f32)

    specs = [
        # out = (-w^5) + C0 w^4 + C1 w^3 + C2 w^2 + C3 w   (Horner, depth 8)
        ("IVE_P5", Spec(
            body=_spill_c3_to_src1(
                ((((C0 - Src0) * Src0 + C1) * Src0 + C2) * Src0 + C3) * Src0),
            reference=ref_p5)),
    ]
    new_ops = []
    for name, spec in specs:
        op = dve_ops.DveOp(name, spec, subdim=False, uops_sha={})
        dve_ops.OPS.append(op)
        new_ops.append(op)
    dve_ops._SUB_OPCODE_FOR_NAME.update(
        {op.name: dve_ops._CUSTOM_DVE_ROW_BASE + i
         for i, op in enumerate(dve_ops.OPS)}
    )
    dve_ops.CUSTOM_DVE_SPECS.update({op.name: op.spec for op in new_ops})
    for op in new_ops:
        shas = {}
        for ver in ("v3", "v4"):
            try:
                s = DveOpSpec(
                    name=op.name,
                    opcode=dve_ops.get_dve_sub_opcode(op.name),
                    uops=lower(op.spec, ver=ver),
                    rd1_en=_has_src1(op.spec),
                )
                shas[ver] = s.sha(ver)
            except Exception:
                pass
        object.__setattr__(op, "uops_sha", shas)
    return {o.name: o for o in dve_ops.OPS}


def _build_nc():
    import concourse.bacc as bacc
    import concourse.tile as tile
    from concourse import mybir
    from contextlib import ExitStack

    ops = _register_ops()
    F16 = mybir.dt.float16
    F32 = mybir.dt.float32
    AF = mybir.ActivationFunctionType
    P5 = ops["IVE_P5"]

    nc = bacc.Bacc(
        "TRN2", target_bir_lowering=False, debug=False,
        enable_asserts=True, num_devices=NCORES,
    )
    z = nc.dram_tensor("z", [NT, P, FD], F16, kind="ExternalInput").ap()
    out = nc.dram_tensor("out", [NT, P, FD], F16, kind="ExternalOutput").ap()

    with tile.TileContext(nc) as tc, ExitStack() as ctx:
        cpool = ctx.enter_context(tc.tile_pool(name="const", bufs=1))
        ctail = cpool.tile([P, 1], F32, tag="ctail")
        nc.vector.memset(ctail[:], _C3)
        bias_t = cpool.tile([P, 1], F32, tag="bias")
        nc.vector.memset(bias_t[:], _BIAS)

        pools = {}
        for name, bufs, dt in [("x", 3, F16), ("w", 2, F32), ("o", 3, F16)]:
            pools[name] = (ctx.enter_context(
                tc.tile_pool(name=name, bufs=bufs)), dt)
        for i in range(NT):
            xp, xdt = pools["x"]
            xt = xp.tile([P, FD], xdt, tag="x")
            nc.sync.dma_start(out=xt[:], in_=z[i])
            wp, wdt = pools["w"]
            wt = wp.tile([P, FD], wdt, tag="w")
            nc.scalar.activation(wt[:], xt[:], AF.Abs_reciprocal_sqrt,
                                 bias=bias_t[:], scale=_SCALE)
            op_, odt = pools["o"]
            ot = op_.tile([P, FD], odt, tag="o")
            nc.vector._custom_dve(P5, out=ot[:], in0=wt[:], in1=ctail[:],
                                  s0=_C0, s1=_C1, imm2=_C2)
            nc.scalar.dma_start(out=out[i], in_=ot[:])
    nc.compile()
    return nc


def _get_nc():
    if "nc" not in _state:
        _state["nc"] = _build_nc()
    return _state["nc"]


def kernel(z: np.ndarray) -> np.ndarray:
    from concourse.bass_utils import run_bass_kernel_spmd

    z = np.asarray(z)
    assert z.shape == FULL_SHAPE, z.shape
    z16 = np.ascontiguousarray(z, dtype=np.float16)
    nc = _get_nc()
    shards = z16.reshape(NCORES, NT, P, FD)
    in_maps = [{"z": shards[i]} for i in range(NCORES)]
    try:
        res = run_bass_kernel_spmd(nc, in_maps, list(range(NCORES)))
    except Exception:
        res = run_bass_kernel_spmd(nc, in_maps, list(range(NCORES)))
    outs = [res.results[i]["out"].astype(np.float32).reshape(PER_CORE)
            for i in range(NCORES)]
    return np.concatenate(outs, axis=0)


# revision 8
# speedup vs baseline: 4.6995x; 4.6995x over previous
"""Trainium2 Bass kernel for elementwise i1e(z) = exp(-|z|) * I1(z),
z in [0.1, 50], shape (32, 4096, 1024) f32, data-parallel over 8 cores.

Fast path (memory-regime):
    HBM I/O in fp16 (halves DMA traffic; tolerance is 2e-2).
    w' = k / sqrt(z + B)      (1 ACT pass: Dsqrt(s*z + s*B) = 0.5/sqrt(.))
    out = ((((C0 - w')*w' + C1)*w' + C2)*w' + C3)*w'
                              (1 custom DVE pass, depth-8, 4 constants;
                               quintic with negative leading coeff -k^5)
Fit: minimax-relative deg-5 odd-free poly of i1e in w = rsqrt(z+B),
B=2.448; max rel err ~6.3e-3 (fit) / ~7e-3 (incl. fp16 quantization).
"""
import numpy as np

NCORES = 8
NT, P, FD = 32, 128, 4096          # per-core: 32 tiles of [128, 4096] fp16
FULL_SHAPE = (32, 4096, 1024)
PER_CORE = (4, 4096, 1024)

# --- approximation constants (see module docstring) ---
_B = 2.4480
_C15 = [0.193748, 3.346473, -19.009253, 47.355799, -41.558003]  # c1..c5
_K = float((-_C15[4]) ** 0.2)                 # 2.10732...
# AbsRsqrt(s*z + s*B) = 1/sqrt(s) * rsqrt(z+B) = k*rsqrt(z+B), k=1/sqrt(s)
_SCALE = float(1.0 / _K**2)
_BIAS = float(_B * _SCALE)
_C0 = float(_C15[3] / _K**4)                  # coeff of w'^4 (s0)
_C1 = float(_C15[2] / _K**3)                  # coeff of w'^3 (s1)
_C2 = float(_C15[1] / _K**2)                  # coeff of w'^2 (imm2)
_C3 = float(_C15[0] / _K)                     # coeff of w'^1 (in1 spill)

_state = {}


def _register_ops():
    import concourse.dve_ops as dve_ops
    from concourse.dve_spec import (
        Spec, Src0, C0, C1, C2, C3, _spill_c3_to_src1, lower, _has_src1,
    )
    from concourse.dve_uop import DveOpSpec

    if "IVE_P5" in dve_ops._SUB_OPCODE_FOR_NAME:
        return {o.name: o for o in dve_ops.OPS}

    f32 = np.float32

    def ref_p5(in0, in1, s0, s1, imm2):
        c3 = np.asarray(in1, f32).reshape(-1, 1)
        x = in0.astype(f32)
        return (((((s0 - x) * x + s1) * x + imm2) * x + c3) * x).astype(f32)

    specs = [
        # out = (-w^5) + C0 w^4 + C1 w^3 + C2 w^2 + C3 w   (Horner, depth 8)
        ("IVE_P5", Spec(
            body=_spill_c3_to_src1(
                ((((C0 - Src0) * Src0 + C1) * Src0 + C2) * Src0 + C3) * Src0),
            reference=ref_p5)),
    ]
    new_ops = []
    for name, spec in specs:
        op = dve_ops.DveOp(name, spec, subdim=False, uops_sha={})
        dve_ops.OPS.append(op)
        new_ops.append(op)
    dve_ops._SUB_OPCODE_FOR_NAME.update(
        {op.name: dve_ops._CUSTOM_DVE_ROW_BASE + i
         for i, op in enumerate(dve_ops.OPS)}
    )
    dve_ops.CUSTOM_DVE_SPECS.update({op.name: op.spec for op in new_ops})
    for op in new_ops:
        shas = {}
        for ver in ("v3", "v4"):
            try:
                s = DveOpSpec(
                    name=op.name,
                    opcode=dve_ops.get_dve_sub_opcode(op.name),
                    uops=lower(op.spec, ver=ver),
                    rd1_en=_has_src1(op.spec),
                )
                shas[ver] = s.sha(ver)
            except Exception:
                pass
        object.__setattr__(op, "uops_sha", shas)
    return {o.name: o for o in dve_ops.OPS}


def _build_nc(reps: int = 1):
    """reps>1 unrolls the whole pass multiple times inside the device
    program (same I/O, identical per-rep work) — used by the timing
    harness to cancel launch overhead: (t_reps - t_1)/(reps-1)."""
    import concourse.bacc as bacc
    import concourse.tile as tile
    from concourse import mybir
    from contextlib import ExitStack

    ops = _register_ops()
    F16 = mybir.dt.float16
    F32 = mybir.dt.float32
    AF = mybir.ActivationFunctionType
    P5 = ops["IVE_P5"]

    nc = bacc.Bacc(
        "TRN2", target_bir_lowering=False, debug=False,
        enable_asserts=True, num_devices=NCORES,
    )
    z = nc.dram_tensor("z", [NT, P, FD], F16, kind="ExternalInput").ap()
    out = nc.dram_tensor("out", [NT, P, FD], F16, kind="ExternalOutput").ap()

    with tile.TileContext(nc) as tc, ExitStack() as ctx:
        cpool = ctx.enter_context(tc.tile_pool(name="const", bufs=1))
        ctail = cpool.tile([P, 1], F32, tag="ctail")
        nc.vector.memset(ctail[:], _C3)
        bias_t = cpool.tile([P, 1], F32, tag="bias")
        nc.vector.memset(bias_t[:], _BIAS)

        pools = {}
        for name, bufs, dt in [("x", 3, F16), ("w", 2, F32), ("o", 3, F16)]:
            pools[name] = (ctx.enter_context(
                tc.tile_pool(name=name, bufs=bufs)), dt)
        for _ in range(reps):
            for i in range(NT):
                xp, xdt = pools["x"]
                xt = xp.tile([P, FD], xdt, tag="x")
                nc.sync.dma_start(out=xt[:], in_=z[i])
                wp, wdt = pools["w"]
                wt = wp.tile([P, FD], wdt, tag="w")
                nc.scalar.activation(wt[:], xt[:], AF.Abs_reciprocal_sqrt,
                                     bias=bias_t[:], scale=_SCALE)
                op_, odt = pools["o"]
                ot = op_.tile([P, FD], odt, tag="o")
                nc.vector._custom_dve(P5, out=ot[:], in0=wt[:], in1=ctail[:],
                                      s0=_C0, s1=_C1, imm2=_C2)
                nc.scalar.dma_start(out=out[i], in_=ot[:])
    nc.compile()
    return nc


def _get_nc():
    if "nc" not in _state:
        _state["nc"] = _build_nc()
    return _state["nc"]


def kernel(z: np.ndarray) -> np.ndarray:
    from concourse.bass_utils import run_bass_kernel_spmd

    z = np.asarray(z)
    assert z.shape == FULL_SHAPE, z.shape
    z16 = np.ascontiguousarray(z, dtype=np.float16)
    nc = _get_nc()
    shards = z16.reshape(NCORES, NT, P, FD)
    in_maps = [{"z": shards[i]} for i in range(NCORES)]
    try:
        res = run_bass_kernel_spmd(nc, in_maps, list(range(NCORES)))
    except Exception:
        res = run_bass_kernel_spmd(nc, in_maps, list(range(NCORES)))
    outs = [res.results[i]["out"].astype(np.float32).reshape(PER_CORE)
            for i in range(NCORES)]
    return np.concatenate(outs, axis=0)


# revision 12
# speedup vs baseline: 10.1061x; 2.1505x over previous
"""Trainium2 Bass kernel for elementwise i1e(z) = exp(-|z|) * I1(z),
z in [0.1, 50], shape (32, 4096, 1024) f32, data-parallel over 8 cores.

Fast path (memory-regime):
    HBM I/O in fp16 (halves DMA traffic; tolerance is 2e-2).
    w' = k / sqrt(z + B)      (1 ACT pass: Dsqrt(s*z + s*B) = 0.5/sqrt(.))
    out = ((((C0 - w')*w' + C1)*w' + C2)*w' + C3)*w'
                              (1 custom DVE pass, depth-8, 4 constants;
                               quintic with negative leading coeff -k^5)
Fit: minimax-relative deg-5 odd-free poly of i1e in w = rsqrt(z+B),
B=2.448; max rel err ~6.3e-3 (fit) / ~7e-3 (incl. fp16 quantization).
"""
import numpy as np

NCORES = 8
NT, P, FD = 64, 128, 2048          # per-core: 64 tiles of [128, 2048] fp16
FULL_SHAPE = (32, 4096, 1024)
PER_CORE = (4, 4096, 1024)

# --- approximation constants (see module docstring) ---
_B = 2.4480
_C15 = [0.193748, 3.346473, -19.009253, 47.355799, -41.558003]  # c1..c5
_K = float((-_C15[4]) ** 0.2)                 # 2.10732...
# AbsRsqrt(s*z + s*B) = 1/sqrt(s) * rsqrt(z+B) = k*rsqrt(z+B), k=1/sqrt(s)
_SCALE = float(1.0 / _K**2)
_BIAS = float(_B * _SCALE)
_C0 = float(_C15[3] / _K**4)                  # coeff of w'^4 (s0)
_C1 = float(_C15[2] / _K**3)                  # coeff of w'^3 (s1)
_C2 = float(_C15[1] / _K**2)                  # coeff of w'^2 (imm2)
_C3 = float(_C15[0] / _K)                     # coeff of w'^1 (in1 spill)

_state = {}


def _register_ops():
    import concourse.dve_ops as dve_ops
    from concourse.dve_spec import (
        Spec, Src0, C0, C1, C2, C3, _spill_c3_to_src1, lower, _has_src1,
    )
    from concourse.dve_uop import DveOpSpec

    if "IVE_P5" in dve_ops._SUB_OPCODE_FOR_NAME:
        return {o.name: o for o in dve_ops.OPS}

    f32 = np.float32

    def ref_p5(in0, in1, s0, s1, imm2):
        c3 = np.asarray(in1, f32).reshape(-1, 1)
        x = in0.astype(f32)
        return (((((s0 - x) * x + s1) * x + imm2) * x + c3) * x).astype(f32)

    specs = [
        # out = (-w^5) + C0 w^4 + C1 w^3 + C2 w^2 + C3 w   (Horner, depth 8)
        ("IVE_P5", Spec(
            body=_spill_c3_to_src1(
                ((((C0 - Src0) * Src0 + C1) * Src0 + C2) * Src0 + C3) * Src0),
            reference=ref_p5)),
    ]
    new_ops = []
    for name, spec in specs:
        op = dve_ops.DveOp(name, spec, subdim=False, uops_sha={})
        dve_ops.OPS.append(op)
        new_ops.append(op)
    dve_ops._SUB_OPCODE_FOR_NAME.update(
        {op.name: dve_ops._CUSTOM_DVE_ROW_BASE + i
         for i, op in enumerate(dve_ops.OPS)}
    )
    dve_ops.CUSTOM_DVE_SPECS.update({op.name: op.spec for op in new_ops})
    for op in new_ops:
        shas = {}
        for ver in ("v3", "v4"):
            try:
                s = DveOpSpec(
                    name=op.name,
                    opcode=dve_ops.get_dve_sub_opcode(op.name),
                    uops=lower(op.spec, ver=ver),
                    rd1_en=_has_src1(op.spec),
                )
                shas[ver] = s.sha(ver)
            except Exception:
                pass
        object.__setattr__(op, "uops_sha", shas)
    return {o.name: o for o in dve_ops.OPS}


def _build_nc(reps: int = 1, skip_out: bool = False):
    """reps>1 unrolls the whole pass multiple times inside the device
    program (same I/O, identical per-rep work) — used by the timing
    harness to cancel launch overhead: (t_reps - t_1)/(reps-1).
    skip_out=True drops all but tile 0's output DMA (bottleneck probe)."""
    import concourse.bacc as bacc
    import concourse.tile as tile
    from concourse import mybir
    from contextlib import ExitStack

    ops = _register_ops()
    F16 = mybir.dt.float16
    F32 = mybir.dt.float32
    AF = mybir.ActivationFunctionType
    P5 = ops["IVE_P5"]

    nc = bacc.Bacc(
        "TRN2", target_bir_lowering=False, debug=False,
        enable_asserts=True, num_devices=NCORES,
    )
    z = nc.dram_tensor("z", [NT, P, FD], F16, kind="ExternalInput").ap()
    out = nc.dram_tensor("out", [NT, P, FD], F16, kind="ExternalOutput").ap()

    with tile.TileContext(nc) as tc, ExitStack() as ctx:
        cpool = ctx.enter_context(tc.tile_pool(name="const", bufs=1))
        ctail = cpool.tile([P, 1], F32, tag="ctail")
        nc.vector.memset(ctail[:], _C3)
        bias_t = cpool.tile([P, 1], F32, tag="bias")
        nc.vector.memset(bias_t[:], _BIAS)

        pools = {}
        for name, bufs, dt in [("x", 6, F16), ("w", 4, F32), ("o", 6, F16)]:
            pools[name] = (ctx.enter_context(
                tc.tile_pool(name=name, bufs=bufs)), dt)
        for _ in range(reps):
            for i in range(NT):
                xp, xdt = pools["x"]
                xt = xp.tile([P, FD], xdt, tag="x")
                nc.sync.dma_start(out=xt[:], in_=z[i])
                wp, wdt = pools["w"]
                wt = wp.tile([P, FD], wdt, tag="w")
                nc.scalar.activation(wt[:], xt[:], AF.Abs_reciprocal_sqrt,
                                     bias=bias_t[:], scale=_SCALE)
                op_, odt = pools["o"]
                ot = op_.tile([P, FD], odt, tag="o")
                nc.vector._custom_dve(P5, out=ot[:], in0=wt[:], in1=ctail[:],
                                      s0=_C0, s1=_C1, imm2=_C2)
                if not (skip_out and i > 0):
                    nc.scalar.dma_start(out=out[i], in_=ot[:])
    nc.compile()
    return nc


def _get_nc():
    if "nc" not in _state:
        _state["nc"] = _build_nc()
    return _state["nc"]


def kernel(z: np.ndarray) -> np.ndarray:
    from concourse.bass_utils import run_bass_kernel_spmd

    z = np.asarray(z)
    assert z.shape == FULL_SHAPE, z.shape
    z16 = np.ascontiguousarray(z, dtype=np.float16)
    nc = _get_nc()
    shards = z16.reshape(NCORES, NT, P, FD)
    in_maps = [{"z": shards[i]} for i in range(NCORES)]
    try:
        res = run_bass_kernel_spmd(nc, in_maps, list(range(NCORES)))
    except Exception:
        res = run_bass_kernel_spmd(nc, in_maps, list(range(NCORES)))
    outs = [res.results[i]["out"].astype(np.float32).reshape(PER_CORE)
            for i in range(NCORES)]
    return np.concatenate(outs, axis=0)
